# revision 23
# baseline (speedup 1.0000x reference)
"""Tensor-parallel multi-head attention kernel for 8 Trainium2 NeuronCores.

Problem: nn_Attention (B=2, S=2048, D=2048, 16 heads x 128) with per-head
RMSNorm on q/k, non-causal softmax attention, and output projection.

Sharding (tensor-parallel over heads, per the hint):
  - core c owns heads {2c, 2c+1}: Wq/Wk/Wv column slices [D, 256], Wo row
    slice [256, D].
  - every core reads all of x (the projection contracts over full D and
    full sequence is needed for non-causal attention keys/values).
  - each core emits a partial output  x-partial = attn_out_c @ Wo_c ; the
    host unshard sums the 8 partials (the natural unshard for row-sharded
    Wo -- equivalent to the all-reduce in the hint, done at gather time).

Device-side layout choices (all matmuls in float32r = full PE rate):
  - host supplies xT = x.T so the contraction dim D lands on SBUF
    partitions; q/k are produced TRANSPOSED ([head_dim, seq]) which is
    exactly the layout attention needs, v is produced as [seq, head_dim].
  - scores are computed transposed (scoresT[kj, qi]) so softmax's
    denominator is a partition-axis sum, done with an all-ones stationary
    matmul that broadcasts the denominator to all 128 partitions.
  - exp on the Scalar engine PSUM->SBUF with the 1/sqrt(dh) scale folded
    into the activation's scale argument; no max-subtraction is needed
    (rmsnorm'd q,k give scores ~N(0,1)).
  - attn@v is computed transposed (outT[d, qi]) so outT directly feeds the
    output projection as the stationary operand.
"""

import math
import sys

for _p in ("/opt/trn_rl_repo",):
    if _p not in sys.path:
        sys.path.insert(0, _p)

import numpy as np

import bass_rust
import concourse.bass as bass
import concourse.mybir as mybir
import concourse.tile as tile

F32 = mybir.dt.float32
F32R = mybir.dt.float32r
AF = mybir.ActivationFunctionType
MUL = mybir.AluOpType.mult
DIV = mybir.AluOpType.divide

N_CORES = 8
N_HEADS = 16
HEAD_DIM = 128
EPS = 1e-6

_wait_counter = [0]


def _split_waits(nc, limit=1):
    """This compiler build rejects >1 semaphore wait per instruction
    ("Too many sync wait commands").  Move excess waits onto preceding
    same-engine no-ops: the sequencer executes them in order, so waiting
    earlier on the same engine is semantically equivalent."""
    for fn in nc.m.functions:
        for blk in fn.blocks:
            newl = []
            changed = False
            for inst in blk.instructions:
                si = inst.sync_info
                waits = list(si.on_wait) if si is not None and si.on_wait else []
                if len(waits) > limit:
                    extra, keep = waits[:-limit], waits[-limit:]
                    for w in extra:
                        _wait_counter[0] += 1
                        nop = bass_rust.InstNoOp(name=f"I-waitsplit-{_wait_counter[0]}")
                        nop.engine = inst.engine
                        nop.sync_info = mybir.SyncInfo(on_wait=[w], on_update=[])
                        newl.append(nop)
                    si.on_wait = keep
                    changed = True
                newl.append(inst)
            if changed:
                blk.instructions = newl


def build_nc(B, S, D, HL, split=True):
    """Emit the per-core program. HL = heads per core."""
    IL = HL * HEAD_DIM          # local inner dim
    NKB = D // 128              # contraction blocks for projections
    SC = 256                    # seq chunk for the projection phase
    NSC = S // SC
    NQB = S // 512              # query blocks in attention
    NKJ = S // 128              # key blocks in attention
    BS = B * S
    scale = 1.0 / math.sqrt(HEAD_DIM)

    nc = bass.Bass("TRN2", target_bir_lowering=False, debug=False,
                   num_devices=N_CORES)
    xT = nc.dram_tensor("xT", [D, BS], F32R, kind="ExternalInput")
    Wq = nc.dram_tensor("Wq", [D, IL], F32R, kind="ExternalInput")
    Wk = nc.dram_tensor("Wk", [D, IL], F32R, kind="ExternalInput")
    Wv = nc.dram_tensor("Wv", [D, IL], F32R, kind="ExternalInput")
    Wo = nc.dram_tensor("Wo", [IL, D], F32R, kind="ExternalInput")
    qg = nc.dram_tensor("qg", [128, 1], F32, kind="ExternalInput")
    kg = nc.dram_tensor("kg", [128, 1], F32, kind="ExternalInput")
    out = nc.dram_tensor("out", [BS, D], F32, kind="ExternalOutput")

    with tile.TileContext(nc) as tc:
        with (
            tc.tile_pool(name="wpool", bufs=1) as wpool,
            tc.tile_pool(name="xpool", bufs=2) as xpool,
            tc.tile_pool(name="qkv", bufs=1) as qkv,
            tc.tile_pool(name="otp", bufs=2) as otp,
            tc.tile_pool(name="tmp", bufs=3) as tmp,
            tc.tile_pool(name="rawp", bufs=6) as rawp,
            tc.tile_pool(name="expool", bufs=6) as expool,
            tc.tile_pool(name="opool", bufs=2) as opool,
            tc.tile_pool(name="psA", bufs=5, space="PSUM") as psA,
            tc.tile_pool(name="psB", bufs=3, space="PSUM") as psB,
        ):
            # ---- resident constants / weights ----
            wq_sb = wpool.tile([128, NKB, IL], F32R, name="wq_sb")
            nc.sync.dma_start(wq_sb[:], Wq.rearrange("(kb p) n -> p kb n", p=128))
            wk_sb = wpool.tile([128, NKB, IL], F32R, name="wk_sb")
            nc.sync.dma_start(wk_sb[:], Wk.rearrange("(kb p) n -> p kb n", p=128))
            wv_sb = wpool.tile([128, NKB, IL], F32R, name="wv_sb")
            nc.sync.dma_start(wv_sb[:], Wv.rearrange("(kb p) n -> p kb n", p=128))
            wo_sb = wpool.tile([128, HL, D], F32R, name="wo_sb")
            gq = wpool.tile([128, 1], F32, name="gq")
            nc.sync.dma_start(gq[:], qg[:])
            gk = wpool.tile([128, 1], F32, name="gk")
            nc.sync.dma_start(gk[:], kg[:])
            ones_f32 = wpool.tile([128, 128], F32, name="ones_f32")
            nc.vector.memset(ones_f32[:], 1.0)
            ones_sb = wpool.tile([128, 128], F32R, name="ones_sb")
            nc.scalar.copy(ones_sb[:], ones_f32[:])
            eps_sb = wpool.tile([128, 1], F32, name="eps_sb")
            nc.vector.memset(eps_sb[:], EPS)

            for b in range(B):
                # ---------------- projection + rmsnorm ----------------
                qt = qkv.tile([128, HL, S], F32R, tag="qt", name=f"qt{b}")
                kt = qkv.tile([128, HL, S], F32R, tag="kt", name=f"kt{b}")
                vt = qkv.tile([128, NKJ, IL], F32R, tag="vt", name=f"vt{b}")
                for sc in range(NSC):
                    off = b * S + sc * SC
                    xt = xpool.tile([128, NKB, SC], F32R, tag="xt",
                                    name=f"xt{b}_{sc}")
                    nc.sync.dma_start(
                        xt[:], xT[:, off:off + SC].rearrange(
                            "(kb p) s -> p kb s", p=128))

                    # q/k passes: psum [dh=128, SC] per head, then rmsnorm.
                    # The psum is copied to SBUF immediately so the bank
                    # recycles fast; the rms chain runs off the PE's path.
                    for pi, (w_sb, dstT, gam, pname) in enumerate((
                        (wq_sb, qt, gq, "q"), (wk_sb, kt, gk, "k"),
                    )):
                        for h in range(HL):
                            idx = pi * HL + h
                            ph = psA.tile([128, SC], F32, tag="psA",
                                             name=f"ps_{pname}{b}_{sc}_{h}")
                            for kb in range(NKB):
                                nc.tensor.matmul(
                                    ph[:], w_sb[:, kb, h * 128:(h + 1) * 128],
                                    xt[:, kb, :],
                                    start=(kb == 0), stop=(kb == NKB - 1))
                            raw = rawp.tile([128, SC], F32, tag="raw",
                                            name=f"raw_{pname}{b}_{sc}_{h}")
                            nc.vector.tensor_copy(raw[:], ph[:])
                            sq = tmp.tile([128, SC], F32R, tag="sq",
                                          name=f"sq_{pname}{b}_{sc}_{h}")
                            if idx % 2 == 0:
                                nc.scalar.activation(sq[:], raw[:], AF.Square)
                            else:
                                nc.vector.tensor_mul(sq[:], raw[:], raw[:])
                            ssum = psB.tile([128, SC], F32, tag="psB",
                                               name=f"ssum_{pname}{b}_{sc}_{h}")
                            nc.tensor.matmul(ssum[:], ones_sb[:], sq[:],
                                             start=True, stop=True)
                            # rstd = (mean(x^2)+eps)^-0.5 = exp(-0.5*ln(.)),
                            # both on ACT (DVE reciprocal is ~5 cyc/elem).
                            lg = tmp.tile([128, SC], F32, tag="lg",
                                          name=f"lg_{pname}{b}_{sc}_{h}")
                            nc.scalar.activation(lg[:], ssum[:], AF.Ln,
                                                 bias=eps_sb[:],
                                                 scale=1.0 / HEAD_DIM)
                            rstd = tmp.tile([128, SC], F32, tag="rstd",
                                            name=f"rstd_{pname}{b}_{sc}_{h}")
                            nc.scalar.activation(rstd[:], lg[:], AF.Exp,
                                                 scale=-0.5)
                            nc.vector.scalar_tensor_tensor(
                                out=dstT[:, h, sc * SC:(sc + 1) * SC],
                                in0=raw[:], scalar=gam[:], in1=rstd[:],
                                op0=MUL, op1=MUL)

                    # v pass: psum [seq 128, IL] per 128-subchunk
                    for j in range(SC // 128):
                        pv = psB.tile([128, IL], F32, tag="psB",
                                         name=f"ps_v{b}_{sc}_{j}")
                        for kb in range(NKB):
                            nc.tensor.matmul(
                                pv[:], xt[:, kb, j * 128:(j + 1) * 128],
                                wv_sb[:, kb, :],
                                start=(kb == 0), stop=(kb == NKB - 1))
                        nc.vector.tensor_copy(vt[:, sc * (SC // 128) + j, :],
                                              pv[:])

                if b == 0:
                    # Wo isn't needed until the first output projection;
                    # loading it here keeps the early DMA queue clear for xt.
                    nc.sync.dma_start(
                        wo_sb[:], Wo.rearrange("(h p) n -> p h n", p=128))

                # -------- attention + fused output projection --------
                # qb outer / h inner so each 512-wide query block finishes
                # both heads and immediately projects through Wo; the
                # partial-output DMA then spreads across the whole phase
                # instead of forming a tail.
                for qb in range(NQB):
                    ot_qb = otp.tile([128, HL, 512], F32R, tag="ot",
                                     name=f"ot{b}_{qb}")
                    for h in range(HL):
                        den = psA.tile([128, 512], F32, tag="psA",
                                          name=f"den{b}_{h}_{qb}")
                        av = psB.tile([128, 512], F32, tag="psB",
                                         name=f"av{b}_{h}_{qb}")
                        for kj in range(NKJ):
                            sc_ps = psA.tile([128, 512], F32, tag="psA",
                                                name=f"sc{b}_{h}_{qb}_{kj}")
                            nc.tensor.matmul(
                                sc_ps[:], kt[:, h, kj * 128:(kj + 1) * 128],
                                qt[:, h, qb * 512:(qb + 1) * 512],
                                start=True, stop=True)
                            ex = expool.tile([128, 512], F32R, tag="ex",
                                             name=f"ex{b}_{h}_{qb}_{kj}")
                            nc.scalar.activation(ex[:], sc_ps[:], AF.Exp,
                                                 scale=scale)
                            nc.tensor.matmul(den[:], ones_sb[:], ex[:],
                                             start=(kj == 0),
                                             stop=(kj == NKJ - 1))
                            nc.tensor.matmul(
                                av[:], vt[:, kj, h * 128:(h + 1) * 128], ex[:],
                                start=(kj == 0), stop=(kj == NKJ - 1))
                        lden = tmp.tile([128, 512], F32, tag="lden",
                                        name=f"lden{b}_{h}_{qb}")
                        nc.scalar.activation(lden[:], den[:], AF.Ln)
                        r = tmp.tile([128, 512], F32, tag="r",
                                     name=f"r{b}_{h}_{qb}")
                        nc.scalar.activation(r[:], lden[:], AF.Exp, scale=-1.0)
                        nc.vector.tensor_mul(ot_qb[:, h, :], av[:], r[:])

                    for qs in range(4):          # 4 x 128 query rows in qb
                        qi = qb * 4 + qs
                        for dc in range(D // 512):
                            pp = psA if (qi * 4 + dc) % 2 == 0 else psB
                            po = pp.tile(
                                [128, 512], F32,
                                tag="psA" if (qi * 4 + dc) % 2 == 0 else "psB",
                                name=f"po{b}_{qi}_{dc}")
                            for h in range(HL):
                                nc.tensor.matmul(
                                    po[:],
                                    ot_qb[:, h, qs * 128:(qs + 1) * 128],
                                    wo_sb[:, h, dc * 512:(dc + 1) * 512],
                                    start=(h == 0), stop=(h == HL - 1))
                            oo = opool.tile([128, 512], F32, tag="oo",
                                            name=f"oo{b}_{qi}_{dc}")
                            if dc % 2 == 0:
                                nc.scalar.copy(oo[:], po[:])
                            else:
                                nc.vector.tensor_copy(oo[:], po[:])
                            nc.sync.dma_start(
                                out[b * S + qi * 128: b * S + (qi + 1) * 128,
                                    dc * 512:(dc + 1) * 512], oo[:])

    if split:
        _split_waits(nc)
    return nc


def _prep_in_maps(inputs, B, S, D, HL):
    """Shard the full inputs for the 8 cores."""
    x = np.asarray(inputs["x"], dtype=np.float32)
    Wq = np.asarray(inputs["Wq"], dtype=np.float32)
    Wk = np.asarray(inputs["Wk"], dtype=np.float32)
    Wv = np.asarray(inputs["Wv"], dtype=np.float32)
    Wo = np.asarray(inputs["Wo"], dtype=np.float32)
    qg = np.ascontiguousarray(
        np.asarray(inputs["q_gamma"], dtype=np.float32).reshape(128, 1))
    kg = np.ascontiguousarray(
        np.asarray(inputs["k_gamma"], dtype=np.float32).reshape(128, 1))
    xTf = np.ascontiguousarray(x.reshape(B * S, D).T)
    IL = HL * HEAD_DIM
    in_maps = []
    for c in range(N_CORES):
        cs = slice(c * IL, (c + 1) * IL)
        in_maps.append({
            "xT": xTf,
            "Wq": np.ascontiguousarray(Wq[:, cs]),
            "Wk": np.ascontiguousarray(Wk[:, cs]),
            "Wv": np.ascontiguousarray(Wv[:, cs]),
            "Wo": np.ascontiguousarray(Wo[cs, :]),
            "qg": qg,
            "kg": kg,
        })
    return in_maps


_NC_CACHE = {}


def run_cores(inputs, trace=False):
    """Build (cached), shard, run on 8 cores; returns (full_out, results)."""
    from concourse.bass_utils import run_bass_kernel_spmd

    x = np.asarray(inputs["x"])
    B, S, D = x.shape
    HL = N_HEADS // N_CORES
    key = (B, S, D, HL)
    if key not in _NC_CACHE:
        _NC_CACHE[key] = build_nc(B, S, D, HL)
    nc = _NC_CACHE[key]
    in_maps = _prep_in_maps(inputs, B, S, D, HL)
    res = run_bass_kernel_spmd(nc, in_maps, list(range(N_CORES)), trace=trace)
    acc = res.results[0]["out"].astype(np.float32)
    for c in range(1, N_CORES):
        acc = acc + res.results[c]["out"]
    return acc.reshape(B, S, D), res


def kernel(**inputs) -> np.ndarray:
    return run_cores(inputs, trace=False)[0]


# revision 24
# speedup vs baseline: 1.0562x; 1.0562x over previous
"""Tensor-parallel multi-head attention kernel for 8 Trainium2 NeuronCores.

Problem: nn_Attention (B=2, S=2048, D=2048, 16 heads x 128) with per-head
RMSNorm on q/k, non-causal softmax attention, and output projection.

Sharding (tensor-parallel over heads, per the hint):
  - core c owns heads {2c, 2c+1}: Wq/Wk/Wv column slices [D, 256], Wo row
    slice [256, D].
  - every core reads all of x (the projection contracts over full D and
    full sequence is needed for non-causal attention keys/values).
  - each core emits a partial output  x-partial = attn_out_c @ Wo_c ; the
    host unshard sums the 8 partials (the natural unshard for row-sharded
    Wo -- equivalent to the all-reduce in the hint, done at gather time).

Device-side layout choices (all matmuls in float32r = full PE rate):
  - host supplies xT = x.T so the contraction dim D lands on SBUF
    partitions; q/k are produced TRANSPOSED ([head_dim, seq]) which is
    exactly the layout attention needs, v is produced as [seq, head_dim].
  - scores are computed transposed (scoresT[kj, qi]) so softmax's
    denominator is a partition-axis sum, done with an all-ones stationary
    matmul that broadcasts the denominator to all 128 partitions.
  - exp on the Scalar engine PSUM->SBUF with the 1/sqrt(dh) scale folded
    into the activation's scale argument; no max-subtraction is needed
    (rmsnorm'd q,k give scores ~N(0,1)).
  - attn@v is computed transposed (outT[d, qi]) so outT directly feeds the
    output projection as the stationary operand.
"""

import math
import sys

for _p in ("/opt/trn_rl_repo",):
    if _p not in sys.path:
        sys.path.insert(0, _p)

import numpy as np

import bass_rust
import concourse.bass as bass
import concourse.mybir as mybir
import concourse.tile as tile

F32 = mybir.dt.float32
F32R = mybir.dt.float32r
AF = mybir.ActivationFunctionType
MUL = mybir.AluOpType.mult
DIV = mybir.AluOpType.divide

N_CORES = 8
N_HEADS = 16
HEAD_DIM = 128
EPS = 1e-6

_wait_counter = [0]


def _split_waits(nc, limit=1):
    """This compiler build rejects >1 semaphore wait per instruction
    ("Too many sync wait commands").  Move excess waits onto preceding
    same-engine no-ops: the sequencer executes them in order, so waiting
    earlier on the same engine is semantically equivalent."""
    for fn in nc.m.functions:
        for blk in fn.blocks:
            newl = []
            changed = False
            for inst in blk.instructions:
                si = inst.sync_info
                waits = list(si.on_wait) if si is not None and si.on_wait else []
                if len(waits) > limit:
                    extra, keep = waits[:-limit], waits[-limit:]
                    for w in extra:
                        _wait_counter[0] += 1
                        nop = bass_rust.InstNoOp(name=f"I-waitsplit-{_wait_counter[0]}")
                        nop.engine = inst.engine
                        nop.sync_info = mybir.SyncInfo(on_wait=[w], on_update=[])
                        newl.append(nop)
                    si.on_wait = keep
                    changed = True
                newl.append(inst)
            if changed:
                blk.instructions = newl


def build_nc(B, S, D, HL, split=True):
    """Emit the per-core program. HL = heads per core."""
    IL = HL * HEAD_DIM          # local inner dim
    NKB = D // 128              # contraction blocks for projections
    SC = 256                    # seq chunk for the projection phase
    NSC = S // SC
    NQB = S // 512              # query blocks in attention
    NKJ = S // 128              # key blocks in attention
    BS = B * S
    scale = 1.0 / math.sqrt(HEAD_DIM)

    nc = bass.Bass("TRN2", target_bir_lowering=False, debug=False,
                   num_devices=N_CORES)
    xT = nc.dram_tensor("xT", [D, BS], F32R, kind="ExternalInput")
    Wq = nc.dram_tensor("Wq", [D, IL], F32R, kind="ExternalInput")
    Wk = nc.dram_tensor("Wk", [D, IL], F32R, kind="ExternalInput")
    Wv = nc.dram_tensor("Wv", [D, IL], F32R, kind="ExternalInput")
    Wo = nc.dram_tensor("Wo", [IL, D], F32R, kind="ExternalInput")
    qg = nc.dram_tensor("qg", [128, 1], F32, kind="ExternalInput")
    kg = nc.dram_tensor("kg", [128, 1], F32, kind="ExternalInput")
    out = nc.dram_tensor("out", [BS, D], F32, kind="ExternalOutput")

    with tile.TileContext(nc) as tc:
        with (
            tc.tile_pool(name="wpool", bufs=1) as wpool,
            tc.tile_pool(name="xpool", bufs=2) as xpool,
            tc.tile_pool(name="qkv", bufs=1) as qkv,
            tc.tile_pool(name="otp", bufs=2) as otp,
            tc.tile_pool(name="tmp", bufs=3) as tmp,
            tc.tile_pool(name="rawp", bufs=6) as rawp,
            tc.tile_pool(name="expool", bufs=6) as expool,
            tc.tile_pool(name="opool", bufs=2) as opool,
            tc.tile_pool(name="psA", bufs=5, space="PSUM") as psA,
            tc.tile_pool(name="psB", bufs=3, space="PSUM") as psB,
        ):
            # ---- resident constants / weights ----
            wq_sb = wpool.tile([128, NKB, IL], F32R, name="wq_sb")
            wk_sb = wpool.tile([128, NKB, IL], F32R, name="wk_sb")
            wv_sb = wpool.tile([128, NKB, IL], F32R, name="wv_sb")
            for kb4 in range(NKB // 4):
                s4 = slice(kb4 * 4 * 128, (kb4 + 1) * 4 * 128)
                for w_sb_, W_ in ((wq_sb, Wq), (wk_sb, Wk), (wv_sb, Wv)):
                    nc.sync.dma_start(
                        w_sb_[:, kb4 * 4:(kb4 + 1) * 4, :],
                        W_[s4, :].rearrange("(kb p) n -> p kb n", p=128))
            wo_sb = wpool.tile([128, HL, D], F32R, name="wo_sb")
            gq = wpool.tile([128, 1], F32, name="gq")
            nc.sync.dma_start(gq[:], qg[:])
            gk = wpool.tile([128, 1], F32, name="gk")
            nc.sync.dma_start(gk[:], kg[:])
            ones_f32 = wpool.tile([128, 128], F32, name="ones_f32")
            nc.vector.memset(ones_f32[:], 1.0)
            ones_sb = wpool.tile([128, 128], F32R, name="ones_sb")
            nc.scalar.copy(ones_sb[:], ones_f32[:])
            eps_sb = wpool.tile([128, 1], F32, name="eps_sb")
            nc.vector.memset(eps_sb[:], EPS)

            for b in range(B):
                # ---------------- projection + rmsnorm ----------------
                qt = qkv.tile([128, HL, S], F32R, tag="qt", name=f"qt{b}")
                kt = qkv.tile([128, HL, S], F32R, tag="kt", name=f"kt{b}")
                vt = qkv.tile([128, NKJ, IL], F32R, tag="vt", name=f"vt{b}")
                for sc in range(NSC):
                    off = b * S + sc * SC
                    xt = xpool.tile([128, NKB, SC], F32R, tag="xt",
                                    name=f"xt{b}_{sc}")
                    for kb4 in range(NKB // 4):
                        nc.sync.dma_start(
                            xt[:, kb4 * 4:(kb4 + 1) * 4, :],
                            xT[kb4 * 4 * 128:(kb4 + 1) * 4 * 128,
                               off:off + SC].rearrange(
                                "(kb p) s -> p kb s", p=128))

                    # q/k passes: psum [dh=128, SC] per head, then rmsnorm.
                    # The psum is copied to SBUF immediately so the bank
                    # recycles fast; the rms chain runs off the PE's path.
                    for pi, (w_sb, dstT, gam, pname) in enumerate((
                        (wq_sb, qt, gq, "q"), (wk_sb, kt, gk, "k"),
                    )):
                        for h in range(HL):
                            idx = pi * HL + h
                            ph = psA.tile([128, SC], F32, tag="psA",
                                             name=f"ps_{pname}{b}_{sc}_{h}")
                            for kb in range(NKB):
                                nc.tensor.matmul(
                                    ph[:], w_sb[:, kb, h * 128:(h + 1) * 128],
                                    xt[:, kb, :],
                                    start=(kb == 0), stop=(kb == NKB - 1))
                            raw = rawp.tile([128, SC], F32, tag="raw",
                                            name=f"raw_{pname}{b}_{sc}_{h}")
                            nc.vector.tensor_copy(raw[:], ph[:])
                            sq = tmp.tile([128, SC], F32R, tag="sq",
                                          name=f"sq_{pname}{b}_{sc}_{h}")
                            if idx % 2 == 0:
                                nc.scalar.activation(sq[:], raw[:], AF.Square)
                            else:
                                nc.vector.tensor_mul(sq[:], raw[:], raw[:])
                            ssum = psB.tile([128, SC], F32, tag="psB",
                                               name=f"ssum_{pname}{b}_{sc}_{h}")
                            nc.tensor.matmul(ssum[:], ones_sb[:], sq[:],
                                             start=True, stop=True)
                            # rstd = (mean(x^2)+eps)^-0.5 = exp(-0.5*ln(.)),
                            # both on ACT (DVE reciprocal is ~5 cyc/elem).
                            lg = tmp.tile([128, SC], F32, tag="lg",
                                          name=f"lg_{pname}{b}_{sc}_{h}")
                            nc.scalar.activation(lg[:], ssum[:], AF.Ln,
                                                 bias=eps_sb[:],
                                                 scale=1.0 / HEAD_DIM)
                            rstd = tmp.tile([128, SC], F32, tag="rstd",
                                            name=f"rstd_{pname}{b}_{sc}_{h}")
                            nc.scalar.activation(rstd[:], lg[:], AF.Exp,
                                                 scale=-0.5)
                            nc.vector.scalar_tensor_tensor(
                                out=dstT[:, h, sc * SC:(sc + 1) * SC],
                                in0=raw[:], scalar=gam[:], in1=rstd[:],
                                op0=MUL, op1=MUL)

                    # v pass: psum [seq 128, IL] per 128-subchunk
                    for j in range(SC // 128):
                        pv = psB.tile([128, IL], F32, tag="psB",
                                         name=f"ps_v{b}_{sc}_{j}")
                        for kb in range(NKB):
                            nc.tensor.matmul(
                                pv[:], xt[:, kb, j * 128:(j + 1) * 128],
                                wv_sb[:, kb, :],
                                start=(kb == 0), stop=(kb == NKB - 1))
                        nc.vector.tensor_copy(vt[:, sc * (SC // 128) + j, :],
                                              pv[:])

                if b == 0:
                    # Wo isn't needed until the first output projection;
                    # loading it here keeps the early DMA queue clear for xt.
                    nc.sync.dma_start(
                        wo_sb[:], Wo.rearrange("(h p) n -> p h n", p=128))

                # -------- attention + fused output projection --------
                # qb outer / h inner so each 512-wide query block finishes
                # both heads and immediately projects through Wo; the
                # partial-output DMA then spreads across the whole phase
                # instead of forming a tail.
                for qb in range(NQB):
                    ot_qb = otp.tile([128, HL, 512], F32R, tag="ot",
                                     name=f"ot{b}_{qb}")
                    for h in range(HL):
                        den = psA.tile([128, 512], F32, tag="psA",
                                          name=f"den{b}_{h}_{qb}")
                        av = psB.tile([128, 512], F32, tag="psB",
                                         name=f"av{b}_{h}_{qb}")
                        for kj in range(NKJ):
                            sc_ps = psA.tile([128, 512], F32, tag="psA",
                                                name=f"sc{b}_{h}_{qb}_{kj}")
                            nc.tensor.matmul(
                                sc_ps[:], kt[:, h, kj * 128:(kj + 1) * 128],
                                qt[:, h, qb * 512:(qb + 1) * 512],
                                start=True, stop=True)
                            ex = expool.tile([128, 512], F32R, tag="ex",
                                             name=f"ex{b}_{h}_{qb}_{kj}")
                            nc.scalar.activation(ex[:], sc_ps[:], AF.Exp,
                                                 scale=scale)
                            nc.tensor.matmul(den[:], ones_sb[:], ex[:],
                                             start=(kj == 0),
                                             stop=(kj == NKJ - 1))
                            nc.tensor.matmul(
                                av[:], vt[:, kj, h * 128:(h + 1) * 128], ex[:],
                                start=(kj == 0), stop=(kj == NKJ - 1))
                        lden = tmp.tile([128, 512], F32, tag="lden",
                                        name=f"lden{b}_{h}_{qb}")
                        nc.scalar.activation(lden[:], den[:], AF.Ln)
                        r = tmp.tile([128, 512], F32, tag="r",
                                     name=f"r{b}_{h}_{qb}")
                        nc.scalar.activation(r[:], lden[:], AF.Exp, scale=-1.0)
                        nc.vector.tensor_mul(ot_qb[:, h, :], av[:], r[:])

                    for qs in range(4):          # 4 x 128 query rows in qb
                        qi = qb * 4 + qs
                        for dc in range(D // 512):
                            pp = psA if (qi * 4 + dc) % 2 == 0 else psB
                            po = pp.tile(
                                [128, 512], F32,
                                tag="psA" if (qi * 4 + dc) % 2 == 0 else "psB",
                                name=f"po{b}_{qi}_{dc}")
                            for h in range(HL):
                                nc.tensor.matmul(
                                    po[:],
                                    ot_qb[:, h, qs * 128:(qs + 1) * 128],
                                    wo_sb[:, h, dc * 512:(dc + 1) * 512],
                                    start=(h == 0), stop=(h == HL - 1))
                            oo = opool.tile([128, 512], F32, tag="oo",
                                            name=f"oo{b}_{qi}_{dc}")
                            if dc % 2 == 0:
                                nc.scalar.copy(oo[:], po[:])
                            else:
                                nc.vector.tensor_copy(oo[:], po[:])
                            nc.sync.dma_start(
                                out[b * S + qi * 128: b * S + (qi + 1) * 128,
                                    dc * 512:(dc + 1) * 512], oo[:])

    if split:
        _split_waits(nc)
    return nc


def _prep_in_maps(inputs, B, S, D, HL):
    """Shard the full inputs for the 8 cores."""
    x = np.asarray(inputs["x"], dtype=np.float32)
    Wq = np.asarray(inputs["Wq"], dtype=np.float32)
    Wk = np.asarray(inputs["Wk"], dtype=np.float32)
    Wv = np.asarray(inputs["Wv"], dtype=np.float32)
    Wo = np.asarray(inputs["Wo"], dtype=np.float32)
    qg = np.ascontiguousarray(
        np.asarray(inputs["q_gamma"], dtype=np.float32).reshape(128, 1))
    kg = np.ascontiguousarray(
        np.asarray(inputs["k_gamma"], dtype=np.float32).reshape(128, 1))
    xTf = np.ascontiguousarray(x.reshape(B * S, D).T)
    IL = HL * HEAD_DIM
    in_maps = []
    for c in range(N_CORES):
        cs = slice(c * IL, (c + 1) * IL)
        in_maps.append({
            "xT": xTf,
            "Wq": np.ascontiguousarray(Wq[:, cs]),
            "Wk": np.ascontiguousarray(Wk[:, cs]),
            "Wv": np.ascontiguousarray(Wv[:, cs]),
            "Wo": np.ascontiguousarray(Wo[cs, :]),
            "qg": qg,
            "kg": kg,
        })
    return in_maps


_NC_CACHE = {}


def run_cores(inputs, trace=False):
    """Build (cached), shard, run on 8 cores; returns (full_out, results)."""
    from concourse.bass_utils import run_bass_kernel_spmd

    x = np.asarray(inputs["x"])
    B, S, D = x.shape
    HL = N_HEADS // N_CORES
    key = (B, S, D, HL)
    if key not in _NC_CACHE:
        _NC_CACHE[key] = build_nc(B, S, D, HL)
    nc = _NC_CACHE[key]
    in_maps = _prep_in_maps(inputs, B, S, D, HL)
    res = run_bass_kernel_spmd(nc, in_maps, list(range(N_CORES)), trace=trace)
    acc = res.results[0]["out"].astype(np.float32)
    for c in range(1, N_CORES):
        acc = acc + res.results[c]["out"]
    return acc.reshape(B, S, D), res


def kernel(**inputs) -> np.ndarray:
    return run_cores(inputs, trace=False)[0]


# revision 25
# speedup vs baseline: 1.1440x; 1.0832x over previous
"""Tensor-parallel multi-head attention kernel for 8 Trainium2 NeuronCores.

Problem: nn_Attention (B=2, S=2048, D=2048, 16 heads x 128) with per-head
RMSNorm on q/k, non-causal softmax attention, and output projection.

Sharding (tensor-parallel over heads, per the hint):
  - core c owns heads {2c, 2c+1}: Wq/Wk/Wv column slices [D, 256], Wo row
    slice [256, D].
  - every core reads all of x (the projection contracts over full D and
    full sequence is needed for non-causal attention keys/values).
  - each core emits a partial output  x-partial = attn_out_c @ Wo_c ; the
    host unshard sums the 8 partials (the natural unshard for row-sharded
    Wo -- equivalent to the all-reduce in the hint, done at gather time).

Device-side layout choices (all matmuls in float32r = full PE rate):
  - host supplies xT = x.T so the contraction dim D lands on SBUF
    partitions; q/k are produced TRANSPOSED ([head_dim, seq]) which is
    exactly the layout attention needs, v is produced as [seq, head_dim].
  - scores are computed transposed (scoresT[kj, qi]) so softmax's
    denominator is a partition-axis sum, done with an all-ones stationary
    matmul that broadcasts the denominator to all 128 partitions.
  - exp on the Scalar engine PSUM->SBUF with the 1/sqrt(dh) scale folded
    into the activation's scale argument; no max-subtraction is needed
    (rmsnorm'd q,k give scores ~N(0,1)).
  - attn@v is computed transposed (outT[d, qi]) so outT directly feeds the
    output projection as the stationary operand.
"""

import math
import sys

for _p in ("/opt/trn_rl_repo",):
    if _p not in sys.path:
        sys.path.insert(0, _p)

import numpy as np

import bass_rust
import concourse.bass as bass
import concourse.mybir as mybir
import concourse.tile as tile

F32 = mybir.dt.float32
F32R = mybir.dt.float32r
AF = mybir.ActivationFunctionType
MUL = mybir.AluOpType.mult
DIV = mybir.AluOpType.divide

N_CORES = 8
N_HEADS = 16
HEAD_DIM = 128
EPS = 1e-6

_wait_counter = [0]


def _split_waits(nc, limit=1):
    """This compiler build rejects >1 semaphore wait per instruction
    ("Too many sync wait commands").  Move excess waits onto preceding
    same-engine no-ops: the sequencer executes them in order, so waiting
    earlier on the same engine is semantically equivalent."""
    for fn in nc.m.functions:
        for blk in fn.blocks:
            newl = []
            changed = False
            for inst in blk.instructions:
                si = inst.sync_info
                waits = list(si.on_wait) if si is not None and si.on_wait else []
                if len(waits) > limit:
                    extra, keep = waits[:-limit], waits[-limit:]
                    for w in extra:
                        _wait_counter[0] += 1
                        nop = bass_rust.InstNoOp(name=f"I-waitsplit-{_wait_counter[0]}")
                        nop.engine = inst.engine
                        nop.sync_info = mybir.SyncInfo(on_wait=[w], on_update=[])
                        newl.append(nop)
                    si.on_wait = keep
                    changed = True
                newl.append(inst)
            if changed:
                blk.instructions = newl


def build_nc(B, S, D, HL, split=True):
    """Emit the per-core program. HL = heads per core."""
    IL = HL * HEAD_DIM          # local inner dim
    NKB = D // 128              # contraction blocks for projections
    SC = 256                    # seq chunk for the projection phase
    NSC = S // SC
    NQB = S // 512              # query blocks in attention
    NKJ = S // 128              # key blocks in attention
    BS = B * S
    scale = 1.0 / math.sqrt(HEAD_DIM)

    nc = bass.Bass("TRN2", target_bir_lowering=False, debug=False,
                   num_devices=N_CORES)
    xT = nc.dram_tensor("xT", [D, BS], F32R, kind="ExternalInput")
    Wq = nc.dram_tensor("Wq", [D, IL], F32R, kind="ExternalInput")
    Wk = nc.dram_tensor("Wk", [D, IL], F32R, kind="ExternalInput")
    Wv = nc.dram_tensor("Wv", [D, IL], F32R, kind="ExternalInput")
    Wo = nc.dram_tensor("Wo", [IL, D], F32R, kind="ExternalInput")
    qg = nc.dram_tensor("qg", [128, 1], F32, kind="ExternalInput")
    kg = nc.dram_tensor("kg", [128, 1], F32, kind="ExternalInput")
    out = nc.dram_tensor("out", [BS, D], F32, kind="ExternalOutput")

    with tile.TileContext(nc) as tc:
        with (
            tc.tile_pool(name="wpool", bufs=1) as wpool,
            tc.tile_pool(name="xpool", bufs=2) as xpool,
            tc.tile_pool(name="qkv", bufs=1) as qkv,
            tc.tile_pool(name="otp", bufs=2) as otp,
            tc.tile_pool(name="tmp", bufs=3) as tmp,
            tc.tile_pool(name="rawp", bufs=6) as rawp,
            tc.tile_pool(name="expool", bufs=6) as expool,
            tc.tile_pool(name="opool", bufs=3) as opool,
            tc.tile_pool(name="psA", bufs=5, space="PSUM") as psA,
            tc.tile_pool(name="psB", bufs=3, space="PSUM") as psB,
        ):
            # ---- resident constants / weights ----
            wq_sb = wpool.tile([128, NKB, IL], F32R, name="wq_sb")
            wk_sb = wpool.tile([128, NKB, IL], F32R, name="wk_sb")
            wv_sb = wpool.tile([128, NKB, IL], F32R, name="wv_sb")
            for kb4 in range(NKB // 4):
                s4 = slice(kb4 * 4 * 128, (kb4 + 1) * 4 * 128)
                for w_sb_, W_ in ((wq_sb, Wq), (wk_sb, Wk), (wv_sb, Wv)):
                    nc.sync.dma_start(
                        w_sb_[:, kb4 * 4:(kb4 + 1) * 4, :],
                        W_[s4, :].rearrange("(kb p) n -> p kb n", p=128))
            wo_sb = wpool.tile([128, HL, D], F32R, name="wo_sb")
            gq = wpool.tile([128, 1], F32, name="gq")
            nc.sync.dma_start(gq[:], qg[:])
            gk = wpool.tile([128, 1], F32, name="gk")
            nc.sync.dma_start(gk[:], kg[:])
            ones_f32 = wpool.tile([128, 128], F32, name="ones_f32")
            nc.vector.memset(ones_f32[:], 1.0)
            ones_sb = wpool.tile([128, 128], F32R, name="ones_sb")
            nc.scalar.copy(ones_sb[:], ones_f32[:])
            eps_sb = wpool.tile([128, 1], F32, name="eps_sb")
            nc.vector.memset(eps_sb[:], EPS)

            for b in range(B):
                # ---------------- projection + rmsnorm ----------------
                qt = qkv.tile([128, HL, S], F32R, tag="qt", name=f"qt{b}")
                kt = qkv.tile([128, HL, S], F32R, tag="kt", name=f"kt{b}")
                vt = qkv.tile([128, NKJ, IL], F32R, tag="vt", name=f"vt{b}")
                for sc in range(NSC):
                    off = b * S + sc * SC
                    xt = xpool.tile([128, NKB, SC], F32R, tag="xt",
                                    name=f"xt{b}_{sc}")
                    for kb4 in range(NKB // 4):
                        nc.sync.dma_start(
                            xt[:, kb4 * 4:(kb4 + 1) * 4, :],
                            xT[kb4 * 4 * 128:(kb4 + 1) * 4 * 128,
                               off:off + SC].rearrange(
                                "(kb p) s -> p kb s", p=128))

                    # q/k passes: psum [dh=128, SC] per head, then rmsnorm.
                    # The psum is copied to SBUF immediately so the bank
                    # recycles fast; the rms chain runs off the PE's path.
                    for pi, (w_sb, dstT, gam, pname) in enumerate((
                        (wq_sb, qt, gq, "q"), (wk_sb, kt, gk, "k"),
                    )):
                        for h in range(HL):
                            idx = pi * HL + h
                            ph = psA.tile([128, SC], F32, tag="psA",
                                             name=f"ps_{pname}{b}_{sc}_{h}")
                            for kb in range(NKB):
                                nc.tensor.matmul(
                                    ph[:], w_sb[:, kb, h * 128:(h + 1) * 128],
                                    xt[:, kb, :],
                                    start=(kb == 0), stop=(kb == NKB - 1))
                            raw = rawp.tile([128, SC], F32, tag="raw",
                                            name=f"raw_{pname}{b}_{sc}_{h}")
                            nc.vector.tensor_copy(raw[:], ph[:])
                            sq = tmp.tile([128, SC], F32R, tag="sq",
                                          name=f"sq_{pname}{b}_{sc}_{h}")
                            if idx % 2 == 0:
                                nc.scalar.activation(sq[:], raw[:], AF.Square)
                            else:
                                nc.vector.tensor_mul(sq[:], raw[:], raw[:])
                            ssum = psB.tile([128, SC], F32, tag="psB",
                                               name=f"ssum_{pname}{b}_{sc}_{h}")
                            nc.tensor.matmul(ssum[:], ones_sb[:], sq[:],
                                             start=True, stop=True)
                            # rstd = (mean(x^2)+eps)^-0.5 = exp(-0.5*ln(.)),
                            # both on ACT (DVE reciprocal is ~5 cyc/elem).
                            lg = tmp.tile([128, SC], F32, tag="lg",
                                          name=f"lg_{pname}{b}_{sc}_{h}")
                            nc.scalar.activation(lg[:], ssum[:], AF.Ln,
                                                 bias=eps_sb[:],
                                                 scale=1.0 / HEAD_DIM)
                            rstd = tmp.tile([128, SC], F32, tag="rstd",
                                            name=f"rstd_{pname}{b}_{sc}_{h}")
                            nc.scalar.activation(rstd[:], lg[:], AF.Exp,
                                                 scale=-0.5)
                            nc.vector.scalar_tensor_tensor(
                                out=dstT[:, h, sc * SC:(sc + 1) * SC],
                                in0=raw[:], scalar=gam[:], in1=rstd[:],
                                op0=MUL, op1=MUL)

                    # v pass: psum [seq 128, IL] per 128-subchunk
                    for j in range(SC // 128):
                        pv = psB.tile([128, IL], F32, tag="psB",
                                         name=f"ps_v{b}_{sc}_{j}")
                        for kb in range(NKB):
                            nc.tensor.matmul(
                                pv[:], xt[:, kb, j * 128:(j + 1) * 128],
                                wv_sb[:, kb, :],
                                start=(kb == 0), stop=(kb == NKB - 1))
                        nc.vector.tensor_copy(vt[:, sc * (SC // 128) + j, :],
                                              pv[:])

                if b == 0:
                    # Wo isn't needed until the first output projection;
                    # loading it here keeps the early DMA queue clear for xt.
                    for h_ in range(HL):
                        for dq in range(2):
                            nc.sync.dma_start(
                                wo_sb[:, h_, dq * (D // 2):(dq + 1) * (D // 2)],
                                Wo[h_ * 128:(h_ + 1) * 128,
                                   dq * (D // 2):(dq + 1) * (D // 2)].rearrange(
                                    "(o p) n -> p (o n)", p=128))

                # -------- attention + fused output projection --------
                # qb outer / h inner so each 512-wide query block finishes
                # both heads and immediately projects through Wo; the
                # partial-output DMA then spreads across the whole phase
                # instead of forming a tail.
                for qb in range(NQB):
                    ot_qb = otp.tile([128, HL, 512], F32R, tag="ot",
                                     name=f"ot{b}_{qb}")
                    for h in range(HL):
                        den = psA.tile([128, 512], F32, tag="psA",
                                          name=f"den{b}_{h}_{qb}")
                        av = psB.tile([128, 512], F32, tag="psB",
                                         name=f"av{b}_{h}_{qb}")
                        for kj in range(NKJ):
                            sc_ps = psA.tile([128, 512], F32, tag="psA",
                                                name=f"sc{b}_{h}_{qb}_{kj}")
                            nc.tensor.matmul(
                                sc_ps[:], kt[:, h, kj * 128:(kj + 1) * 128],
                                qt[:, h, qb * 512:(qb + 1) * 512],
                                start=True, stop=True)
                            ex = expool.tile([128, 512], F32R, tag="ex",
                                             name=f"ex{b}_{h}_{qb}_{kj}")
                            nc.scalar.activation(ex[:], sc_ps[:], AF.Exp,
                                                 scale=scale)
                            nc.tensor.matmul(den[:], ones_sb[:], ex[:],
                                             start=(kj == 0),
                                             stop=(kj == NKJ - 1))
                            nc.tensor.matmul(
                                av[:], vt[:, kj, h * 128:(h + 1) * 128], ex[:],
                                start=(kj == 0), stop=(kj == NKJ - 1))
                        lden = tmp.tile([128, 512], F32, tag="lden",
                                        name=f"lden{b}_{h}_{qb}")
                        nc.scalar.activation(lden[:], den[:], AF.Ln)
                        r = tmp.tile([128, 512], F32, tag="r",
                                     name=f"r{b}_{h}_{qb}")
                        nc.scalar.activation(r[:], lden[:], AF.Exp, scale=-1.0)
                        nc.vector.tensor_mul(ot_qb[:, h, :], av[:], r[:])

                    for qs in range(4):          # 4 x 128 query rows in qb
                        qi = qb * 4 + qs
                        for dc in range(D // 512):
                            pp = psA if (qi * 4 + dc) % 2 == 0 else psB
                            po = pp.tile(
                                [128, 512], F32,
                                tag="psA" if (qi * 4 + dc) % 2 == 0 else "psB",
                                name=f"po{b}_{qi}_{dc}")
                            for h in range(HL):
                                nc.tensor.matmul(
                                    po[:],
                                    ot_qb[:, h, qs * 128:(qs + 1) * 128],
                                    wo_sb[:, h, dc * 512:(dc + 1) * 512],
                                    start=(h == 0), stop=(h == HL - 1))
                            oo = opool.tile([128, 512], F32, tag="oo",
                                            name=f"oo{b}_{qi}_{dc}")
                            if dc % 2 == 0:
                                nc.scalar.copy(oo[:], po[:])
                            else:
                                nc.vector.tensor_copy(oo[:], po[:])
                            nc.sync.dma_start(
                                out[b * S + qi * 128: b * S + (qi + 1) * 128,
                                    dc * 512:(dc + 1) * 512], oo[:])

    if split:
        _split_waits(nc)
    return nc


def _prep_in_maps(inputs, B, S, D, HL):
    """Shard the full inputs for the 8 cores."""
    x = np.asarray(inputs["x"], dtype=np.float32)
    Wq = np.asarray(inputs["Wq"], dtype=np.float32)
    Wk = np.asarray(inputs["Wk"], dtype=np.float32)
    Wv = np.asarray(inputs["Wv"], dtype=np.float32)
    Wo = np.asarray(inputs["Wo"], dtype=np.float32)
    qg = np.ascontiguousarray(
        np.asarray(inputs["q_gamma"], dtype=np.float32).reshape(128, 1))
    kg = np.ascontiguousarray(
        np.asarray(inputs["k_gamma"], dtype=np.float32).reshape(128, 1))
    xTf = np.ascontiguousarray(x.reshape(B * S, D).T)
    IL = HL * HEAD_DIM
    in_maps = []
    for c in range(N_CORES):
        cs = slice(c * IL, (c + 1) * IL)
        in_maps.append({
            "xT": xTf,
            "Wq": np.ascontiguousarray(Wq[:, cs]),
            "Wk": np.ascontiguousarray(Wk[:, cs]),
            "Wv": np.ascontiguousarray(Wv[:, cs]),
            "Wo": np.ascontiguousarray(Wo[cs, :]),
            "qg": qg,
            "kg": kg,
        })
    return in_maps


_NC_CACHE = {}


def run_cores(inputs, trace=False):
    """Build (cached), shard, run on 8 cores; returns (full_out, results)."""
    from concourse.bass_utils import run_bass_kernel_spmd

    x = np.asarray(inputs["x"])
    B, S, D = x.shape
    HL = N_HEADS // N_CORES
    key = (B, S, D, HL)
    if key not in _NC_CACHE:
        _NC_CACHE[key] = build_nc(B, S, D, HL)
    nc = _NC_CACHE[key]
    in_maps = _prep_in_maps(inputs, B, S, D, HL)
    res = run_bass_kernel_spmd(nc, in_maps, list(range(N_CORES)), trace=trace)
    acc = res.results[0]["out"].astype(np.float32)
    for c in range(1, N_CORES):
        acc = acc + res.results[c]["out"]
    return acc.reshape(B, S, D), res


def kernel(**inputs) -> np.ndarray:
    return run_cores(inputs, trace=False)[0]
